# revision 15
# baseline (speedup 1.0000x reference)
"""Trainium2 Bass kernel for nn_AttentionLayer (pooling attention).

Computes, for each batch b and head i:
    own  = inputs[b,i,:] @ W1_own[i]                  # [64]
    ev   = inputs[b,j,:] @ W1_ev[i]                   # [j,64]
    h    = relu(own + ev + b1[i])                     # [j,64]
    s    = h @ W2[i]  (+ b2[i], softmax-invariant)    # [j]
    w    = softmax_j(s)
    out[b,i] = sum_j w[j] * inputs[b,j]

Sharding: data-parallel over batch across 8 NeuronCores (256 batches/core).
All parameters are replicated; no collectives.

Self-contained: hardcodes shapes; only needs /opt/trn_rl_repo on sys.path.
"""

import os
import sys
from contextlib import ExitStack

import numpy as np

if "/opt/trn_rl_repo" not in sys.path:
    sys.path.insert(0, "/opt/trn_rl_repo")
os.environ.setdefault("MYCRO_LOCAL_CACHE", "1")

import concourse.bass as bass  # noqa: E402
import concourse.mybir as mybir  # noqa: E402
import concourse.tile as tile  # noqa: E402
from concourse import bacc  # noqa: E402
from concourse import bass_utils  # noqa: E402

# Problem shapes (hardcoded per spec)
B, NINS, D, H = 2048, 16, 768, 64
NCORES = 8
BC = B // NCORES          # 256 batches per core
R = BC * NINS             # 4096 rows (b,j) per core
KT = D // 128             # 6 contraction k-tiles
MT = NINS // 2            # 8 m-tiles of (i,h): tile t holds heads 2t, 2t+1
NCH = 8                   # column chunks per core
CHUNK = R // NCH          # 512 (b,j) columns per chunk
CB = CHUNK // NINS        # 32 batches per chunk

F32 = mybir.dt.float32
F32R = mybir.dt.float32r

_CACHED_NC = None
LAST_RESULTS = None


def _r(ap):
    """Bitcast an fp32 AP to float32r for fast-mode PE matmuls."""
    return ap.bitcast(F32R)


def build_nc():
    nc = bacc.Bacc("TRN2", target_bir_lowering=False, debug=False,
                   num_devices=NCORES)

    x_d = nc.dram_tensor("x", [R, D], F32R, kind="ExternalInput").ap()
    w1ev_d = nc.dram_tensor("w1ev", [128, KT, NINS * H], F32R,
                            kind="ExternalInput").ap()
    w1ow_d = nc.dram_tensor("w1ow", [128, KT, NINS * H], F32R,
                            kind="ExternalInput").ap()
    w2b_d = nc.dram_tensor("w2blk", [128, MT, NINS], F32R,
                           kind="ExternalInput").ap()
    b1ht_d = nc.dram_tensor("b1ht", [H, NINS], F32,
                            kind="ExternalInput").ap()
    e01_d = nc.dram_tensor("e01", [H, 2, 128], F32R,
                           kind="ExternalInput").ap()
    msk_d = nc.dram_tensor("bdmask", [128, 128], F32,
                           kind="ExternalInput").ap()
    idn_d = nc.dram_tensor("ident", [128, 128], F32R, kind="ExternalInput").ap()
    out_d = nc.dram_tensor("out", [R, D], F32, kind="ExternalOutput").ap()

    with tile.TileContext(nc) as tc, ExitStack() as ctx:
        const = ctx.enter_context(tc.tile_pool(name="const", bufs=1))
        xp = ctx.enter_context(tc.tile_pool(name="xp", bufs=10))
        xtp = ctx.enter_context(tc.tile_pool(name="xtp", bufs=2))
        hprep = ctx.enter_context(tc.tile_pool(name="hprep", bufs=3))
        hp = ctx.enter_context(tc.tile_pool(name="hp", bufs=10))
        smp = ctx.enter_context(tc.tile_pool(name="smp", bufs=2))
        ownp = ctx.enter_context(tc.tile_pool(name="ownp", bufs=2))
        o128p = ctx.enter_context(tc.tile_pool(name="o128p", bufs=2))
        wtp = ctx.enter_context(tc.tile_pool(name="wtp", bufs=3))
        bdp = ctx.enter_context(tc.tile_pool(name="bdp", bufs=4))
        outp = ctx.enter_context(tc.tile_pool(name="outp", bufs=4))
        # PSUM pools (8 banks total):
        trps = ctx.enter_context(tc.tile_pool(name="trps", bufs=2,
                                              space="PSUM"))  # 2 banks
        evps = ctx.enter_context(tc.tile_pool(name="evps", bufs=2,
                                              space="PSUM"))  # 2 banks
        smallps = ctx.enter_context(tc.tile_pool(name="smallps", bufs=2,
                                                 space="PSUM"))  # 2 banks
        poolps = ctx.enter_context(tc.tile_pool(name="poolps", bufs=2,
                                                space="PSUM"))  # 2 banks

        # --- constants ---
        w1ev_sb = const.tile([128, KT, NINS * H], F32, tag="w1ev")
        nc.sync.dma_start(_r(w1ev_sb[:]), w1ev_d[:])
        w1ow_sb = const.tile([128, KT, NINS * H], F32, tag="w1ow")
        nc.sync.dma_start(_r(w1ow_sb[:]), w1ow_d[:])
        w2b_sb = const.tile([128, MT, NINS], F32, tag="w2b")
        nc.sync.dma_start(_r(w2b_sb[:]), w2b_d[:])
        b1ht_sb = const.tile([H, NINS], F32, tag="b1ht")
        nc.sync.dma_start(b1ht_sb[:], b1ht_d[:])
        e01_sb = const.tile([H, 2, 128], F32, tag="e01")
        nc.sync.dma_start(_r(e01_sb[:]), e01_d[:])
        msk_sb = const.tile([128, 128], F32, tag="msk")
        nc.sync.dma_start(msk_sb[:], msk_d[:])
        idn_sb = const.tile([128, 128], F32, tag="idn")
        nc.sync.dma_start(_r(idn_sb[:]), idn_d[:])

        for c in range(NCH):
            # ---- load X rows for this chunk (used for transpose AND pooling)
            xc = []
            for rt in range(4):
                t_ = xp.tile([128, D], F32, tag="xc")
                nc.sync.dma_start(
                    _r(t_[:]),
                    x_d[c * CHUNK + rt * 128: c * CHUNK + (rt + 1) * 128, :])
                xc.append(t_)

            # ---- transpose X chunk: xt[d, (b,j)] ----
            xt = xtp.tile([128, KT, CHUNK], F32, tag="xt")
            for k in range(KT):
                tp = trps.tile([128, CHUNK], F32, tag="trp")
                for rt in range(4):
                    nc.tensor.transpose(
                        _r(tp[:, rt * 128:(rt + 1) * 128]),
                        _r(xc[rt][:, k * 128:(k + 1) * 128]),
                        _r(idn_sb[:]),
                    )
                nc.scalar.copy(_r(xt[:, k, :]), tp[:])

            # ---- own contribution: own[h, i, b] at partitions 0:64 ----
            op_ = smallps.tile([H, CHUNK], F32, tag="small")
            for i in range(NINS):
                for k in range(KT):
                    nc.tensor.matmul(
                        op_[:, i * CB:(i + 1) * CB],
                        lhsT=_r(w1ow_sb[:, k, i * H:(i + 1) * H]),
                        rhs=_r(xt[:, k, i::NINS]),
                        start=(k == 0), stop=(k == KT - 1),
                    )
            own_sb = ownp.tile([H, NINS, CB], F32, tag="own")
            # own + b1 (b1ht[h,i] broadcast over b); output f32r (feeds lift)
            nc.vector.scalar_tensor_tensor(
                _r(own_sb)[:],
                op_.rearrange("p (i b) -> p i b", b=CB),
                0.0,
                b1ht_sb[:, :, None].to_broadcast([H, NINS, CB]),
                mybir.AluOpType.add,
                mybir.AluOpType.add,
            )
            # ---- lift to 128 partitions via E0/E1: own128[(il,h), t, b]
            lp = trps.tile([128, CHUNK], F32, tag="trp")
            nc.tensor.matmul(lp[:, :MT * CB], lhsT=_r(e01_sb[:, 0, :]),
                             rhs=_r(own_sb[:, 0::2, :]),
                             start=True, stop=False)
            nc.tensor.matmul(lp[:, :MT * CB], lhsT=_r(e01_sb[:, 1, :]),
                             rhs=_r(own_sb[:, 1::2, :]),
                             start=False, stop=True)
            own128 = o128p.tile([128, MT, CB], F32, tag="own128")
            nc.vector.tensor_copy(own128[:],
                                  lp[:, :MT * CB].rearrange(
                                      "p (t b) -> p t b", b=CB))

            # ---- evidence matmuls + h = relu(ev + own + b1) ----
            hts = []
            for t in range(MT):
                evp_t = evps.tile([128, CHUNK], F32, tag="ev")
                for k in range(KT):
                    nc.tensor.matmul(
                        evp_t[:],
                        lhsT=_r(w1ev_sb[:, k, t * 128:(t + 1) * 128]),
                        rhs=_r(xt[:, k, :]),
                        start=(k == 0), stop=(k == KT - 1),
                    )
                hpre = hprep.tile([128, CB, NINS], F32, tag="hpre")
                nc.vector.tensor_tensor(
                    hpre[:],
                    evp_t.rearrange("p (b j) -> p b j", j=NINS),
                    own128[:, t, :, None].to_broadcast([128, CB, NINS]),
                    mybir.AluOpType.add,
                )
                h_t = hp.tile([128, CHUNK], F32, tag="h")
                nc.gpsimd.tensor_scalar_max(
                    _r(h_t[:]), hpre.rearrange("p b j -> p (b j)"), 0.0)
                hts.append(h_t)

            # ---- scores[i, (b,j)] accumulated over m-tiles ----
            scp = smallps.tile([H, CHUNK], F32, tag="small")
            for t in range(MT):
                nc.tensor.matmul(
                    scp[:NINS, :],
                    lhsT=_r(w2b_sb[:, t, :]),
                    rhs=_r(hts[t][:]),
                    start=(t == 0), stop=(t == MT - 1),
                )

            # ---- softmax over j ----
            scv = scp[:NINS, :].rearrange("p (b j) -> p b j", j=NINS)
            mx = smp.tile([NINS, CB], F32, tag="mx")
            nc.vector.tensor_reduce(mx[:], scv, axis=mybir.AxisListType.X,
                                    op=mybir.AluOpType.max)
            esub = smp.tile([NINS, CB, NINS], F32, tag="esub")
            nc.vector.tensor_tensor(
                esub[:], scv, mx[:, :, None].to_broadcast([NINS, CB, NINS]),
                mybir.AluOpType.subtract)
            ex = smp.tile([NINS, CB, NINS], F32, tag="ex")
            nc.scalar.activation(ex[:], esub[:],
                                 mybir.ActivationFunctionType.Exp)
            ssum = smp.tile([NINS, CB], F32, tag="ssum")
            nc.vector.tensor_reduce(ssum[:], ex[:], axis=mybir.AxisListType.X,
                                    op=mybir.AluOpType.add)
            rinv = smp.tile([NINS, CB], F32, tag="rinv")
            nc.vector.reciprocal(rinv[:], ssum[:])
            wgt = smp.tile([NINS, CHUNK], F32, tag="wgt")
            nc.vector.tensor_tensor(
                _r(wgt).rearrange("p (b j) -> p b j", j=NINS),
                ex[:], rinv[:, :, None].to_broadcast([NINS, CB, NINS]),
                mybir.AluOpType.mult)

            # ---- pooling: out[(b,i), d] = sum_j w[b,i,j] * x[(b,j), d] ----
            for rt in range(4):
                tp2 = trps.tile([128, CHUNK], F32, tag="trp")
                nc.tensor.transpose(
                    _r(tp2[:, :NINS]),
                    _r(wgt[:, rt * 128:(rt + 1) * 128]),
                    _r(idn_sb[:NINS, :NINS]),
                )
                wt_sb = wtp.tile([128, NINS], F32, tag="wt")
                nc.vector.tensor_copy(wt_sb[:], tp2[:, :NINS])
                bd = bdp.tile([128, 8, NINS], F32, tag="bd")
                nc.vector.tensor_tensor(
                    _r(bd[:]),
                    wt_sb[:, None, :].to_broadcast([128, 8, NINS]),
                    msk_sb.rearrange("p (g i) -> p g i", i=NINS),
                    mybir.AluOpType.mult)
                bdf = bd.rearrange("p g i -> p (g i)")
                pp_a = poolps.tile([128, 384], F32, tag="pool")
                pp_b = poolps.tile([128, 384], F32, tag="pool")
                nc.tensor.matmul(pp_a[:], lhsT=_r(bdf), rhs=_r(xc[rt][:, :384]),
                                 start=True, stop=True)
                nc.tensor.matmul(pp_b[:], lhsT=_r(bdf), rhs=_r(xc[rt][:, 384:]),
                                 start=True, stop=True)
                osb = outp.tile([128, D], F32, tag="osb")
                nc.scalar.copy(osb[:, :384], pp_a[:])
                nc.scalar.copy(osb[:, 384:], pp_b[:])
                nc.sync.dma_start(
                    out_d[c * CHUNK + rt * 128: c * CHUNK + (rt + 1) * 128, :],
                    osb[:])

    nc.compile()
    return nc


def host_prep(W1, b1, W2):
    """Build the replicated parameter tensors (numpy, fp32)."""
    W1 = np.asarray(W1, dtype=np.float32)
    b1 = np.asarray(b1, dtype=np.float32)
    W2 = np.asarray(W2, dtype=np.float32)
    W1o, W1e = W1[:, :D, :], W1[:, D:, :]

    def to_ktiles(w):  # [16, 768, 64] -> [128, 6, 1024] (cols i*64+h)
        return np.ascontiguousarray(
            w.transpose(1, 0, 2).reshape(KT, 128, NINS * H).transpose(1, 0, 2))

    w1ev = to_ktiles(W1e)
    w1ow = to_ktiles(W1o)
    w2blk = np.zeros((128, MT, NINS), dtype=np.float32)
    for t in range(MT):
        for il in range(2):
            i = 2 * t + il
            w2blk[il * H:(il + 1) * H, t, i] = W2[i]
    b1ht = np.ascontiguousarray(b1.T)
    e01 = np.zeros((H, 2, 128), dtype=np.float32)
    for k in range(H):
        e01[k, 0, k] = 1.0
        e01[k, 1, H + k] = 1.0
    p = np.arange(128)
    bdmask = (p[:, None] // NINS == p[None, :] // NINS).astype(np.float32)
    ident = np.eye(128, dtype=np.float32)
    return dict(w1ev=w1ev, w1ow=w1ow, w2blk=w2blk, b1ht=b1ht, e01=e01,
                bdmask=bdmask, ident=ident)


def get_nc():
    global _CACHED_NC
    if _CACHED_NC is None:
        _CACHED_NC = build_nc()
    return _CACHED_NC


def make_in_maps(inputs, W1, b1, W2):
    consts = host_prep(W1, b1, W2)
    inputs = np.asarray(inputs, dtype=np.float32)
    in_maps = []
    for core in range(NCORES):
        shard = np.ascontiguousarray(
            inputs[core * BC:(core + 1) * BC].reshape(R, D))
        m = dict(consts)
        m["x"] = shard
        in_maps.append(m)
    return in_maps


def kernel(inputs, W1, b1, W2, b2, trace=False):
    """Full-input entry point: shards over 8 cores, returns full output."""
    global LAST_RESULTS
    nc = get_nc()
    in_maps = make_in_maps(inputs, W1, b1, W2)
    res = bass_utils.run_bass_kernel_spmd(
        nc, in_maps, core_ids=list(range(NCORES)), trace=trace)
    LAST_RESULTS = res
    out = np.concatenate(
        [np.asarray(r["out"]).reshape(BC, NINS, D) for r in res.results],
        axis=0)
    return out.astype(np.float32)


if __name__ == "__main__":
    if "--build" in sys.argv:
        get_nc()
        print("build OK")


# revision 16
# speedup vs baseline: 2.0393x; 2.0393x over previous
"""Trainium2 Bass kernel for nn_AttentionLayer (pooling attention).

Computes, for each batch b and head i:
    own  = inputs[b,i,:] @ W1_own[i]                  # [64]
    ev   = inputs[b,j,:] @ W1_ev[i]                   # [j,64]
    h    = relu(own + ev + b1[i])                     # [j,64]
    s    = h @ W2[i]  (+ b2[i], softmax-invariant)    # [j]
    w    = softmax_j(s)
    out[b,i] = sum_j w[j] * inputs[b,j]

Sharding: data-parallel over batch across 8 NeuronCores (256 batches/core).
All parameters are replicated; no collectives.

Self-contained: hardcodes shapes; only needs /opt/trn_rl_repo on sys.path.
"""

import os
import sys
from contextlib import ExitStack

import numpy as np

if "/opt/trn_rl_repo" not in sys.path:
    sys.path.insert(0, "/opt/trn_rl_repo")
os.environ.setdefault("MYCRO_LOCAL_CACHE", "1")

import concourse.bass as bass  # noqa: E402
import concourse.mybir as mybir  # noqa: E402
import concourse.tile as tile  # noqa: E402
from concourse import bacc  # noqa: E402
from concourse import bass_utils  # noqa: E402

# Problem shapes (hardcoded per spec)
B, NINS, D, H = 2048, 16, 768, 64
NCORES = 8
BC = B // NCORES          # 256 batches per core
R = BC * NINS             # 4096 rows (b,j) per core
KT = D // 128             # 6 contraction k-tiles
MT = NINS // 2            # 8 m-tiles of (i,h): tile t holds heads 2t, 2t+1
NCH = 8                   # column chunks per core
CHUNK = R // NCH          # 512 (b,j) columns per chunk
CB = CHUNK // NINS        # 32 batches per chunk

F32 = mybir.dt.float32
F32R = mybir.dt.float32r

_CACHED_NC = None
LAST_RESULTS = None


def _r(ap):
    """Bitcast an fp32 AP to float32r for fast-mode PE matmuls."""
    return ap.bitcast(F32R)


def build_nc():
    nc = bacc.Bacc("TRN2", target_bir_lowering=False, debug=False,
                   num_devices=NCORES)

    x_d = nc.dram_tensor("x", [R, D], F32R, kind="ExternalInput").ap()
    w1ev_d = nc.dram_tensor("w1ev", [128, KT, NINS * H], F32R,
                            kind="ExternalInput").ap()
    w1ow_d = nc.dram_tensor("w1ow", [128, KT, NINS * H], F32R,
                            kind="ExternalInput").ap()
    w2b_d = nc.dram_tensor("w2blk", [128, MT, NINS], F32R,
                           kind="ExternalInput").ap()
    b1ht_d = nc.dram_tensor("b1ht", [H, NINS], F32,
                            kind="ExternalInput").ap()
    e01_d = nc.dram_tensor("e01", [H, 2, 128], F32R,
                           kind="ExternalInput").ap()
    msk_d = nc.dram_tensor("bdmask", [128, 128], F32,
                           kind="ExternalInput").ap()
    idn_d = nc.dram_tensor("ident", [128, 128], F32R, kind="ExternalInput").ap()
    out_d = nc.dram_tensor("out", [R, D], F32, kind="ExternalOutput").ap()

    with tile.TileContext(nc) as tc, ExitStack() as ctx:
        const = ctx.enter_context(tc.tile_pool(name="const", bufs=1))
        xp = ctx.enter_context(tc.tile_pool(name="xp", bufs=10))
        xtp = ctx.enter_context(tc.tile_pool(name="xtp", bufs=2))
        hprep = ctx.enter_context(tc.tile_pool(name="hprep", bufs=3))
        hp = ctx.enter_context(tc.tile_pool(name="hp", bufs=10))
        smp = ctx.enter_context(tc.tile_pool(name="smp", bufs=2))
        ownp = ctx.enter_context(tc.tile_pool(name="ownp", bufs=2))
        o128p = ctx.enter_context(tc.tile_pool(name="o128p", bufs=2))
        wtp = ctx.enter_context(tc.tile_pool(name="wtp", bufs=3))
        bdp = ctx.enter_context(tc.tile_pool(name="bdp", bufs=4))
        outp = ctx.enter_context(tc.tile_pool(name="outp", bufs=4))
        # PSUM pools (8 banks total):
        trps = ctx.enter_context(tc.tile_pool(name="trps", bufs=2,
                                              space="PSUM"))  # 2 banks
        evps = ctx.enter_context(tc.tile_pool(name="evps", bufs=2,
                                              space="PSUM"))  # 2 banks
        smallps = ctx.enter_context(tc.tile_pool(name="smallps", bufs=2,
                                                 space="PSUM"))  # 2 banks
        poolps = ctx.enter_context(tc.tile_pool(name="poolps", bufs=2,
                                                space="PSUM"))  # 2 banks

        # --- constants ---
        w1ev_sb = const.tile([128, KT, NINS * H], F32, tag="w1ev")
        nc.sync.dma_start(_r(w1ev_sb[:]), w1ev_d[:])
        w1ow_sb = const.tile([128, KT, NINS * H], F32, tag="w1ow")
        nc.sync.dma_start(_r(w1ow_sb[:]), w1ow_d[:])
        w2b_sb = const.tile([128, MT, NINS], F32, tag="w2b")
        nc.sync.dma_start(_r(w2b_sb[:]), w2b_d[:])
        b1ht_sb = const.tile([H, NINS], F32, tag="b1ht")
        nc.sync.dma_start(b1ht_sb[:], b1ht_d[:])
        e01_sb = const.tile([H, 2, 128], F32, tag="e01")
        nc.sync.dma_start(_r(e01_sb[:]), e01_d[:])
        msk_sb = const.tile([128, 128], F32, tag="msk")
        nc.sync.dma_start(msk_sb[:], msk_d[:])
        idn_sb = const.tile([128, 128], F32, tag="idn")
        nc.sync.dma_start(_r(idn_sb[:]), idn_d[:])

        for c in range(NCH):
            # ---- load X rows for this chunk (used for transpose AND pooling)
            xc = []
            for rt in range(4):
                t_ = xp.tile([128, D], F32, tag="xc")
                nc.sync.dma_start(
                    _r(t_[:]),
                    x_d[c * CHUNK + rt * 128: c * CHUNK + (rt + 1) * 128, :])
                xc.append(t_)

            # ---- transpose X chunk: xt[d, (b,j)] ----
            xt = xtp.tile([128, KT, CHUNK], F32, tag="xt")
            for k in range(KT):
                tp = trps.tile([128, CHUNK], F32, tag="trp")
                for rt in range(4):
                    nc.tensor.transpose(
                        _r(tp[:, rt * 128:(rt + 1) * 128]),
                        _r(xc[rt][:, k * 128:(k + 1) * 128]),
                        _r(idn_sb[:]),
                    )
                nc.scalar.copy(_r(xt[:, k, :]), tp[:])

            # ---- own contribution: own[h, i, b] at partitions 0:64 ----
            op_ = smallps.tile([H, CHUNK], F32, tag="small")
            for i in range(NINS):
                for k in range(KT):
                    nc.tensor.matmul(
                        op_[:, i * CB:(i + 1) * CB],
                        lhsT=_r(w1ow_sb[:, k, i * H:(i + 1) * H]),
                        rhs=_r(xt[:, k, i::NINS]),
                        start=(k == 0), stop=(k == KT - 1),
                    )
            own_sb = ownp.tile([H, NINS, CB], F32, tag="own")
            # own + b1 (b1ht[h,i] broadcast over b); output f32r (feeds lift)
            nc.vector.scalar_tensor_tensor(
                _r(own_sb)[:],
                op_.rearrange("p (i b) -> p i b", b=CB),
                0.0,
                b1ht_sb[:, :, None].to_broadcast([H, NINS, CB]),
                mybir.AluOpType.add,
                mybir.AluOpType.add,
            )
            # ---- lift to 128 partitions via E0/E1: own128[(il,h), t, b]
            lp = trps.tile([128, CHUNK], F32, tag="trp")
            nc.tensor.matmul(lp[:, :MT * CB], lhsT=_r(e01_sb[:, 0, :]),
                             rhs=_r(own_sb[:, 0::2, :]),
                             start=True, stop=False)
            nc.tensor.matmul(lp[:, :MT * CB], lhsT=_r(e01_sb[:, 1, :]),
                             rhs=_r(own_sb[:, 1::2, :]),
                             start=False, stop=True)
            own128 = o128p.tile([128, MT, CB], F32, tag="own128")
            nc.vector.tensor_copy(own128[:],
                                  lp[:, :MT * CB].rearrange(
                                      "p (t b) -> p t b", b=CB))

            # ---- evidence matmuls + h = relu(ev + own + b1) ----
            hts = []
            for t in range(MT):
                evp_t = evps.tile([128, CHUNK], F32, tag="ev")
                for k in range(KT):
                    nc.tensor.matmul(
                        evp_t[:],
                        lhsT=_r(w1ev_sb[:, k, t * 128:(t + 1) * 128]),
                        rhs=_r(xt[:, k, :]),
                        start=(k == 0), stop=(k == KT - 1),
                    )
                hpre = hprep.tile([128, CB, NINS], F32, tag="hpre")
                nc.vector.tensor_tensor(
                    hpre[:],
                    evp_t.rearrange("p (b j) -> p b j", j=NINS),
                    own128[:, t, :, None].to_broadcast([128, CB, NINS]),
                    mybir.AluOpType.add,
                )
                h_t = hp.tile([128, CHUNK], F32, tag="h")
                nc.vector.tensor_scalar_max(
                    _r(h_t[:]), hpre.rearrange("p b j -> p (b j)"), 0.0)
                hts.append(h_t)

            # ---- scores[i, (b,j)] accumulated over m-tiles ----
            scp = smallps.tile([H, CHUNK], F32, tag="small")
            for t in range(MT):
                nc.tensor.matmul(
                    scp[:NINS, :],
                    lhsT=_r(w2b_sb[:, t, :]),
                    rhs=_r(hts[t][:]),
                    start=(t == 0), stop=(t == MT - 1),
                )

            # ---- softmax over j ----
            scv = scp[:NINS, :].rearrange("p (b j) -> p b j", j=NINS)
            mx = smp.tile([NINS, CB], F32, tag="mx")
            nc.vector.tensor_reduce(mx[:], scv, axis=mybir.AxisListType.X,
                                    op=mybir.AluOpType.max)
            esub = smp.tile([NINS, CB, NINS], F32, tag="esub")
            nc.vector.tensor_tensor(
                esub[:], scv, mx[:, :, None].to_broadcast([NINS, CB, NINS]),
                mybir.AluOpType.subtract)
            ex = smp.tile([NINS, CB, NINS], F32, tag="ex")
            nc.scalar.activation(ex[:], esub[:],
                                 mybir.ActivationFunctionType.Exp)
            ssum = smp.tile([NINS, CB], F32, tag="ssum")
            nc.vector.tensor_reduce(ssum[:], ex[:], axis=mybir.AxisListType.X,
                                    op=mybir.AluOpType.add)
            rinv = smp.tile([NINS, CB], F32, tag="rinv")
            nc.vector.reciprocal(rinv[:], ssum[:])
            wgt = smp.tile([NINS, CHUNK], F32, tag="wgt")
            nc.vector.tensor_tensor(
                _r(wgt).rearrange("p (b j) -> p b j", j=NINS),
                ex[:], rinv[:, :, None].to_broadcast([NINS, CB, NINS]),
                mybir.AluOpType.mult)

            # ---- pooling: out[(b,i), d] = sum_j w[b,i,j] * x[(b,j), d] ----
            for rt in range(4):
                tp2 = trps.tile([128, CHUNK], F32, tag="trp")
                nc.tensor.transpose(
                    _r(tp2[:, :NINS]),
                    _r(wgt[:, rt * 128:(rt + 1) * 128]),
                    _r(idn_sb[:NINS, :NINS]),
                )
                wt_sb = wtp.tile([128, NINS], F32, tag="wt")
                nc.vector.tensor_copy(wt_sb[:], tp2[:, :NINS])
                bd = bdp.tile([128, 8, NINS], F32, tag="bd")
                nc.vector.tensor_tensor(
                    _r(bd[:]),
                    wt_sb[:, None, :].to_broadcast([128, 8, NINS]),
                    msk_sb.rearrange("p (g i) -> p g i", i=NINS),
                    mybir.AluOpType.mult)
                bdf = bd.rearrange("p g i -> p (g i)")
                pp_a = poolps.tile([128, 384], F32, tag="pool")
                pp_b = poolps.tile([128, 384], F32, tag="pool")
                nc.tensor.matmul(pp_a[:], lhsT=_r(bdf), rhs=_r(xc[rt][:, :384]),
                                 start=True, stop=True)
                nc.tensor.matmul(pp_b[:], lhsT=_r(bdf), rhs=_r(xc[rt][:, 384:]),
                                 start=True, stop=True)
                osb = outp.tile([128, D], F32, tag="osb")
                nc.scalar.copy(osb[:, :384], pp_a[:])
                nc.scalar.copy(osb[:, 384:], pp_b[:])
                nc.sync.dma_start(
                    out_d[c * CHUNK + rt * 128: c * CHUNK + (rt + 1) * 128, :],
                    osb[:])

    nc.compile()
    return nc


def host_prep(W1, b1, W2):
    """Build the replicated parameter tensors (numpy, fp32)."""
    W1 = np.asarray(W1, dtype=np.float32)
    b1 = np.asarray(b1, dtype=np.float32)
    W2 = np.asarray(W2, dtype=np.float32)
    W1o, W1e = W1[:, :D, :], W1[:, D:, :]

    def to_ktiles(w):  # [16, 768, 64] -> [128, 6, 1024] (cols i*64+h)
        return np.ascontiguousarray(
            w.transpose(1, 0, 2).reshape(KT, 128, NINS * H).transpose(1, 0, 2))

    w1ev = to_ktiles(W1e)
    w1ow = to_ktiles(W1o)
    w2blk = np.zeros((128, MT, NINS), dtype=np.float32)
    for t in range(MT):
        for il in range(2):
            i = 2 * t + il
            w2blk[il * H:(il + 1) * H, t, i] = W2[i]
    b1ht = np.ascontiguousarray(b1.T)
    e01 = np.zeros((H, 2, 128), dtype=np.float32)
    for k in range(H):
        e01[k, 0, k] = 1.0
        e01[k, 1, H + k] = 1.0
    p = np.arange(128)
    bdmask = (p[:, None] // NINS == p[None, :] // NINS).astype(np.float32)
    ident = np.eye(128, dtype=np.float32)
    return dict(w1ev=w1ev, w1ow=w1ow, w2blk=w2blk, b1ht=b1ht, e01=e01,
                bdmask=bdmask, ident=ident)


def get_nc():
    global _CACHED_NC
    if _CACHED_NC is None:
        _CACHED_NC = build_nc()
    return _CACHED_NC


def make_in_maps(inputs, W1, b1, W2):
    consts = host_prep(W1, b1, W2)
    inputs = np.asarray(inputs, dtype=np.float32)
    in_maps = []
    for core in range(NCORES):
        shard = np.ascontiguousarray(
            inputs[core * BC:(core + 1) * BC].reshape(R, D))
        m = dict(consts)
        m["x"] = shard
        in_maps.append(m)
    return in_maps


def kernel(inputs, W1, b1, W2, b2, trace=False):
    """Full-input entry point: shards over 8 cores, returns full output."""
    global LAST_RESULTS
    nc = get_nc()
    in_maps = make_in_maps(inputs, W1, b1, W2)
    res = bass_utils.run_bass_kernel_spmd(
        nc, in_maps, core_ids=list(range(NCORES)), trace=trace)
    LAST_RESULTS = res
    out = np.concatenate(
        [np.asarray(r["out"]).reshape(BC, NINS, D) for r in res.results],
        axis=0)
    return out.astype(np.float32)


if __name__ == "__main__":
    if "--build" in sys.argv:
        get_nc()
        print("build OK")


# revision 18
# speedup vs baseline: 2.4633x; 1.2079x over previous
"""Trainium2 Bass kernel for nn_AttentionLayer (pooling attention).

Computes, for each batch b and head i:
    own  = inputs[b,i,:] @ W1_own[i]                  # [64]
    ev   = inputs[b,j,:] @ W1_ev[i]                   # [j,64]
    h    = relu(own + ev + b1[i])                     # [j,64]
    s    = h @ W2[i]  (+ b2[i], softmax-invariant)    # [j]
    w    = softmax_j(s)
    out[b,i] = sum_j w[j] * inputs[b,j]

Sharding: data-parallel over batch across 8 NeuronCores (256 batches/core).
All parameters are replicated; no collectives.

Self-contained: hardcodes shapes; only needs /opt/trn_rl_repo on sys.path.
"""

import os
import sys
from contextlib import ExitStack

import numpy as np

if "/opt/trn_rl_repo" not in sys.path:
    sys.path.insert(0, "/opt/trn_rl_repo")
os.environ.setdefault("MYCRO_LOCAL_CACHE", "1")

import concourse.bass as bass  # noqa: E402
import concourse.mybir as mybir  # noqa: E402
import concourse.tile as tile  # noqa: E402
from concourse import bacc  # noqa: E402
from concourse import bass_utils  # noqa: E402

# Problem shapes (hardcoded per spec)
B, NINS, D, H = 2048, 16, 768, 64
NCORES = 8
BC = B // NCORES          # 256 batches per core
R = BC * NINS             # 4096 rows (b,j) per core
KT = D // 128             # 6 contraction k-tiles
MT = NINS // 2            # 8 m-tiles of (i,h): tile t holds heads 2t, 2t+1
NCH = 8                   # column chunks per core
CHUNK = R // NCH          # 512 (b,j) columns per chunk
CB = CHUNK // NINS        # 32 batches per chunk

F32 = mybir.dt.float32
F32R = mybir.dt.float32r

_CACHED_NC = None
LAST_RESULTS = None


def _r(ap):
    """Bitcast an fp32 AP to float32r for fast-mode PE matmuls."""
    return ap.bitcast(F32R)


def build_nc():
    nc = bacc.Bacc("TRN2", target_bir_lowering=False, debug=False,
                   num_devices=NCORES)

    x_d = nc.dram_tensor("x", [R, D], F32R, kind="ExternalInput").ap()
    w1ev_d = nc.dram_tensor("w1ev", [128, KT, NINS * H], F32R,
                            kind="ExternalInput").ap()
    w1ow_d = nc.dram_tensor("w1ow", [128, KT, NINS * H], F32R,
                            kind="ExternalInput").ap()
    w2b_d = nc.dram_tensor("w2blk", [128, MT, NINS], F32R,
                           kind="ExternalInput").ap()
    b1ht_d = nc.dram_tensor("b1ht", [H, NINS], F32,
                            kind="ExternalInput").ap()
    e01_d = nc.dram_tensor("e01", [H, 2, 128], F32R,
                           kind="ExternalInput").ap()
    msk_d = nc.dram_tensor("bdmask", [128, 128], F32,
                           kind="ExternalInput").ap()
    idn_d = nc.dram_tensor("ident", [128, 128], F32R, kind="ExternalInput").ap()
    out_d = nc.dram_tensor("out", [R, D], F32, kind="ExternalOutput").ap()

    with tile.TileContext(nc) as tc, ExitStack() as ctx:
        const = ctx.enter_context(tc.tile_pool(name="const", bufs=1))
        xp = ctx.enter_context(tc.tile_pool(name="xp", bufs=10))
        xtp = ctx.enter_context(tc.tile_pool(name="xtp", bufs=2))
        hprep = ctx.enter_context(tc.tile_pool(name="hprep", bufs=3))
        hp = ctx.enter_context(tc.tile_pool(name="hp", bufs=10))
        smp = ctx.enter_context(tc.tile_pool(name="smp", bufs=2))
        ownp = ctx.enter_context(tc.tile_pool(name="ownp", bufs=2))
        o128p = ctx.enter_context(tc.tile_pool(name="o128p", bufs=2))
        wtp = ctx.enter_context(tc.tile_pool(name="wtp", bufs=3))
        bdp = ctx.enter_context(tc.tile_pool(name="bdp", bufs=4))
        outp = ctx.enter_context(tc.tile_pool(name="outp", bufs=4))
        # PSUM pools (8 banks total):
        trps = ctx.enter_context(tc.tile_pool(name="trps", bufs=2,
                                              space="PSUM"))  # 2 banks
        evps = ctx.enter_context(tc.tile_pool(name="evps", bufs=2,
                                              space="PSUM"))  # 2 banks
        smallps = ctx.enter_context(tc.tile_pool(name="smallps", bufs=1,
                                                 space="PSUM"))  # 2 banks
        poolps = ctx.enter_context(tc.tile_pool(name="poolps", bufs=2,
                                                space="PSUM"))  # 2 banks

        # --- constants ---
        w1ev_sb = const.tile([128, KT, NINS * H], F32, tag="w1ev")
        nc.sync.dma_start(_r(w1ev_sb[:]), w1ev_d[:])
        w1ow_sb = const.tile([128, KT, NINS * H], F32, tag="w1ow")
        nc.sync.dma_start(_r(w1ow_sb[:]), w1ow_d[:])
        w2b_sb = const.tile([128, MT, NINS], F32, tag="w2b")
        nc.sync.dma_start(_r(w2b_sb[:]), w2b_d[:])
        b1ht_sb = const.tile([H, NINS], F32, tag="b1ht")
        nc.sync.dma_start(b1ht_sb[:], b1ht_d[:])
        e01_sb = const.tile([H, 2, 128], F32, tag="e01")
        nc.sync.dma_start(_r(e01_sb[:]), e01_d[:])
        msk_sb = const.tile([128, 128], F32, tag="msk")
        nc.sync.dma_start(msk_sb[:], msk_d[:])
        idn_sb = const.tile([128, 128], F32, tag="idn")
        nc.sync.dma_start(_r(idn_sb[:]), idn_d[:])

        PB = 2 * CB  # 64 batches per chunk pair

        def do_softmax(scp):
            scv = scp[:NINS, :].rearrange("p (b j) -> p b j", j=NINS)
            mx = smp.tile([NINS, CB], F32, tag="mx")
            nc.vector.tensor_reduce(mx[:], scv, axis=mybir.AxisListType.X,
                                    op=mybir.AluOpType.max)
            esub = smp.tile([NINS, CB, NINS], F32, tag="esub")
            nc.vector.tensor_tensor(
                esub[:], scv, mx[:, :, None].to_broadcast([NINS, CB, NINS]),
                mybir.AluOpType.subtract)
            ex = smp.tile([NINS, CB, NINS], F32, tag="ex")
            nc.scalar.activation(ex[:], esub[:],
                                 mybir.ActivationFunctionType.Exp)
            ssum = smp.tile([NINS, CB], F32, tag="ssum")
            nc.vector.tensor_reduce(ssum[:], ex[:], axis=mybir.AxisListType.X,
                                    op=mybir.AluOpType.add)
            rinv = smp.tile([NINS, CB], F32, tag="rinv")
            nc.vector.reciprocal(rinv[:], ssum[:])
            wgt = smp.tile([NINS, CHUNK], F32, tag="wgt")
            nc.vector.tensor_tensor(
                _r(wgt).rearrange("p (b j) -> p b j", j=NINS),
                ex[:], rinv[:, :, None].to_broadcast([NINS, CB, NINS]),
                mybir.AluOpType.mult)
            return wgt

        def do_pool(c, wgt, xc):
            for rt in range(4):
                tp2 = trps.tile([128, CHUNK], F32, tag="trp")
                nc.tensor.transpose(
                    _r(tp2[:, :NINS]),
                    _r(wgt[:, rt * 128:(rt + 1) * 128]),
                    _r(idn_sb[:NINS, :NINS]),
                )
                wt_sb = wtp.tile([128, NINS], F32, tag="wt")
                nc.vector.tensor_copy(wt_sb[:], tp2[:, :NINS])
                bd = bdp.tile([128, 8, NINS], F32, tag="bd")
                nc.vector.tensor_tensor(
                    _r(bd[:]),
                    wt_sb[:, None, :].to_broadcast([128, 8, NINS]),
                    msk_sb.rearrange("p (g i) -> p g i", i=NINS),
                    mybir.AluOpType.mult)
                bdf = bd.rearrange("p g i -> p (g i)")
                pp_a = poolps.tile([128, 384], F32, tag="pool")
                pp_b = poolps.tile([128, 384], F32, tag="pool")
                nc.tensor.matmul(pp_a[:], lhsT=_r(bdf),
                                 rhs=_r(xc[rt][:, :384]),
                                 start=True, stop=True)
                nc.tensor.matmul(pp_b[:], lhsT=_r(bdf),
                                 rhs=_r(xc[rt][:, 384:]),
                                 start=True, stop=True)
                osb = outp.tile([128, D], F32, tag="osb")
                nc.scalar.copy(osb[:, :384], pp_a[:])
                nc.scalar.copy(osb[:, 384:], pp_b[:])
                nc.sync.dma_start(
                    out_d[c * CHUNK + rt * 128: c * CHUNK + (rt + 1) * 128, :],
                    osb[:])

        for p in range(NCH // 2):
            # ---- load + transpose both chunks of the pair ----
            xt = xtp.tile([128, KT, 2 * CHUNK], F32, tag="xt")
            xcs = []
            for parity in range(2):
                c = 2 * p + parity
                xc = []
                for rt in range(4):
                    t_ = xp.tile([128, D], F32, tag="xc")
                    nc.sync.dma_start(
                        _r(t_[:]),
                        x_d[c * CHUNK + rt * 128:
                            c * CHUNK + (rt + 1) * 128, :])
                    xc.append(t_)
                xcs.append(xc)
                for k in range(KT):
                    tp = trps.tile([128, CHUNK], F32, tag="trp")
                    for rt in range(4):
                        nc.tensor.transpose(
                            _r(tp[:, rt * 128:(rt + 1) * 128]),
                            _r(xc[rt][:, k * 128:(k + 1) * 128]),
                            _r(idn_sb[:]),
                        )
                    nc.scalar.copy(
                        _r(xt[:, k, parity * CHUNK:(parity + 1) * CHUNK]),
                        tp[:])

            # ---- own for the pair (N=64): own[h, i, b64] ----
            op_ = smallps.tile([H, NINS * PB], F32, tag="small")
            for i in range(NINS):
                for k in range(KT):
                    nc.tensor.matmul(
                        op_[:, i * PB:(i + 1) * PB],
                        lhsT=_r(w1ow_sb[:, k, i * H:(i + 1) * H]),
                        rhs=_r(xt[:, k, i::NINS]),
                        start=(k == 0), stop=(k == KT - 1),
                    )
            own_sb = ownp.tile([H, NINS, PB], F32, tag="own")
            nc.vector.scalar_tensor_tensor(
                _r(own_sb)[:],
                op_.rearrange("p (i b) -> p i b", b=PB),
                0.0,
                b1ht_sb[:, :, None].to_broadcast([H, NINS, PB]),
                mybir.AluOpType.add,
                mybir.AluOpType.add,
            )
            # lift to 128 partitions via E0/E1: own128[(il,h), t, b64]
            lp = trps.tile([128, CHUNK], F32, tag="trp")
            nc.tensor.matmul(lp[:, :MT * PB], lhsT=_r(e01_sb[:, 0, :]),
                             rhs=_r(own_sb[:, 0::2, :]),
                             start=True, stop=False)
            nc.tensor.matmul(lp[:, :MT * PB], lhsT=_r(e01_sb[:, 1, :]),
                             rhs=_r(own_sb[:, 1::2, :]),
                             start=False, stop=True)
            own128 = o128p.tile([128, MT, PB], F32, tag="own128")
            nc.vector.tensor_copy(own128[:],
                                  lp[:, :MT * PB].rearrange(
                                      "p (t b) -> p t b", b=PB))

            # ---- per chunk: EV + relu + scores + softmax ----
            wgts = []
            for parity in range(2):
                c = 2 * p + parity
                hts = []
                for t in range(MT):
                    evp_t = evps.tile([128, CHUNK], F32, tag="ev")
                    for k in range(KT):
                        nc.tensor.matmul(
                            evp_t[:],
                            lhsT=_r(w1ev_sb[:, k, t * 128:(t + 1) * 128]),
                            rhs=_r(xt[:, k,
                                      parity * CHUNK:(parity + 1) * CHUNK]),
                            start=(k == 0), stop=(k == KT - 1),
                        )
                    hpre = hprep.tile([128, CB, NINS], F32, tag="hpre")
                    nc.vector.tensor_tensor(
                        hpre[:],
                        evp_t.rearrange("p (b j) -> p b j", j=NINS),
                        own128[:, t, parity * CB:(parity + 1) * CB, None]
                        .to_broadcast([128, CB, NINS]),
                        mybir.AluOpType.add,
                    )
                    h_t = hp.tile([128, CHUNK], F32, tag="h")
                    nc.vector.tensor_scalar_max(
                        _r(h_t[:]), hpre.rearrange("p b j -> p (b j)"), 0.0)
                    hts.append(h_t)
                scp = smallps.tile([H, NINS * PB], F32, tag="small")
                for t in range(MT):
                    nc.tensor.matmul(
                        scp[:NINS, :CHUNK],
                        lhsT=_r(w2b_sb[:, t, :]),
                        rhs=_r(hts[t][:]),
                        start=(t == 0), stop=(t == MT - 1),
                    )
                wgts.append(do_softmax(scp[:, :CHUNK]))

            # ---- pooling for both chunks ----
            do_pool(2 * p, wgts[0], xcs[0])
            do_pool(2 * p + 1, wgts[1], xcs[1])

    nc.compile()
    return nc


def host_prep(W1, b1, W2):
    """Build the replicated parameter tensors (numpy, fp32)."""
    W1 = np.asarray(W1, dtype=np.float32)
    b1 = np.asarray(b1, dtype=np.float32)
    W2 = np.asarray(W2, dtype=np.float32)
    W1o, W1e = W1[:, :D, :], W1[:, D:, :]

    def to_ktiles(w):  # [16, 768, 64] -> [128, 6, 1024] (cols i*64+h)
        return np.ascontiguousarray(
            w.transpose(1, 0, 2).reshape(KT, 128, NINS * H).transpose(1, 0, 2))

    w1ev = to_ktiles(W1e)
    w1ow = to_ktiles(W1o)
    w2blk = np.zeros((128, MT, NINS), dtype=np.float32)
    for t in range(MT):
        for il in range(2):
            i = 2 * t + il
            w2blk[il * H:(il + 1) * H, t, i] = W2[i]
    b1ht = np.ascontiguousarray(b1.T)
    e01 = np.zeros((H, 2, 128), dtype=np.float32)
    for k in range(H):
        e01[k, 0, k] = 1.0
        e01[k, 1, H + k] = 1.0
    p = np.arange(128)
    bdmask = (p[:, None] // NINS == p[None, :] // NINS).astype(np.float32)
    ident = np.eye(128, dtype=np.float32)
    return dict(w1ev=w1ev, w1ow=w1ow, w2blk=w2blk, b1ht=b1ht, e01=e01,
                bdmask=bdmask, ident=ident)


def get_nc():
    global _CACHED_NC
    if _CACHED_NC is None:
        _CACHED_NC = build_nc()
    return _CACHED_NC


def make_in_maps(inputs, W1, b1, W2):
    consts = host_prep(W1, b1, W2)
    inputs = np.asarray(inputs, dtype=np.float32)
    in_maps = []
    for core in range(NCORES):
        shard = np.ascontiguousarray(
            inputs[core * BC:(core + 1) * BC].reshape(R, D))
        m = dict(consts)
        m["x"] = shard
        in_maps.append(m)
    return in_maps


def kernel(inputs, W1, b1, W2, b2, trace=False):
    """Full-input entry point: shards over 8 cores, returns full output."""
    global LAST_RESULTS
    nc = get_nc()
    in_maps = make_in_maps(inputs, W1, b1, W2)
    res = bass_utils.run_bass_kernel_spmd(
        nc, in_maps, core_ids=list(range(NCORES)), trace=trace)
    LAST_RESULTS = res
    out = np.concatenate(
        [np.asarray(r["out"]).reshape(BC, NINS, D) for r in res.results],
        axis=0)
    return out.astype(np.float32)


if __name__ == "__main__":
    if "--build" in sys.argv:
        get_nc()
        print("build OK")
